# revision 1
# baseline (speedup 1.0000x reference)
"""MinLSTM cell for Trainium2 (Bass/Tile), v3: engine-balanced bf16.

Data-parallel over batch on 8 cores (one row per core). Profile-driven
design (measured: DVE STT=687ns, scan=1367ns, ACT=721ns, Pool TT=1156ns,
DMA-issue=650ns on the issuing engine, fp8 DoubleRow = 2x only):

  - PE: three bf16 projections (f, i, h) per [128,TC] tile — 18 matmuls,
    bf16 halves LDWEIGHTS vs f32r. fp8 tricks don't pay: the z+/z-
    recombination costs more DVE time than the PE time saved.
  - gates via Exp/Ln from the single ACT table (4 ACT ops/tile):
    ef=Exp(-zf-bf) straight from PSUM, ei likewise, ln2=Ln(ef+ei+2)
    (the +2 via a memset bias tile), rt=Exp(-ln2)=1/(2+Ef+Ei).
  - Pool (idle otherwise, PSUM-illegal, only TensorTensor lowers):
    s2=ef+ei, t1=ei*rt, at=t1+rt.
  - DVE (the scarce engine): ut=(ef+1)*rt, bt=(psh+bh)*ut, scan. 2.7us
    vs 5.5us in the fp8 variant.
  - three-stage software pipeline (lag 0/1/2) so no in-order engine
    queue ever waits mid-chain; output DMAs issue from Sync, not Pool.
  - output hT [H,T] bf16; host converts/transposes.
"""

import sys

if "/opt/trn_rl_repo" not in sys.path:
    sys.path.insert(0, "/opt/trn_rl_repo")

import numpy as np
import ml_dtypes

B, T, D, H = 8, 4096, 768, 768
TC = 512                    # time-chunk (one PSUM bank of fp32)
NT = T // TC                # 8 chunks
KD = D // 128               # 6 bf16 K-tiles
MH = H // 128               # 6 hidden tiles

_state = {}


def _build():
    import concourse.mybir as mybir
    import concourse.tile as tile
    from concourse import bacc

    f32 = mybir.dt.float32
    bf16 = mybir.dt.bfloat16
    A = mybir.AluOpType
    Act = mybir.ActivationFunctionType

    nc = bacc.Bacc("TRN2", target_bir_lowering=False, debug=False, num_devices=B)

    xh_d = nc.dram_tensor("xh", [NT, 128, KD, TC], bf16, kind="ExternalInput")
    w_d = {p: nc.dram_tensor(f"w{p}", [KD, 128, H], bf16, kind="ExternalInput") for p in "fih"}
    b_d = {p: nc.dram_tensor(f"b{p}", [128, MH], f32, kind="ExternalInput") for p in "fih"}
    h0_d = nc.dram_tensor("h0c", [128, MH], f32, kind="ExternalInput")
    hT = nc.dram_tensor("hT", [H, T], f32, kind="ExternalOutput")

    with tile.TileContext(nc) as tc:
        with (
            tc.tile_pool(name="wpool", bufs=1) as wpool,
            tc.tile_pool(name="cpool", bufs=1) as cpool,
            tc.tile_pool(name="xpool", bufs=2) as xpool,
            tc.tile_pool(name="pspool", bufs=8, space="PSUM") as pspool,
            tc.tile_pool(name="wk", bufs=4) as wk,
            tc.tile_pool(name="hpool", bufs=3) as hpool,
        ):
            # Head: chunk-0 x first on sync, split by kd so the first
            # matmul only waits for its own k-slice; each projection's
            # weights on their own issue queue, f first.
            xht0 = xpool.tile([128, KD, TC], bf16, tag="xh", name="xh_0")
            for kd in range(KD):
                nc.sync.dma_start(xht0[:, kd, :], xh_d[0, :, kd, :])
            w_sb = {}
            w_q = {"f": nc.gpsimd, "i": nc.scalar, "h": nc.sync}
            for p in "fih":
                w_sb[p] = wpool.tile([128, KD, H], bf16, tag=f"w{p}", name=f"w{p}s")
                for kd in range(KD):
                    w_q[p].dma_start(w_sb[p][:, kd, :], w_d[p][kd])
            b_sb = {}
            for p in "fih":
                b_sb[p] = cpool.tile([128, MH], f32, tag=f"b{p}", name=f"bs{p}")
                nc.gpsimd.dma_start(b_sb[p][:], b_d[p][:])
            h0_sb = cpool.tile([128, MH], f32, tag="h0")
            nc.gpsimd.dma_start(h0_sb[:], h0_d[:])
            two_sb = cpool.tile([128, 1], f32, tag="two")
            nc.gpsimd.memset(two_sb[:], 2.0)

            prev_h = [None] * MH
            pending = []
            pending2 = []

            def emit_proj(c, j, ps, key, xht):
                # psf/psi drain fast (stage1 ACT); psh lives until stage3's
                # bt two groups later — give it its own 3-bank rotation so
                # the PE never stalls on a bank held by the gate pipeline.
                tag, nb = ("psh", 3) if key == "h" else ("psfi", 5)
                pt = pspool.tile([128, TC], f32, tag=tag, bufs=nb, name=f"p{key}{c}_{j}")
                for kd in range(KD):
                    nc.tensor.matmul(
                        pt[:],
                        w_sb[key][:, kd, j * 128:(j + 1) * 128],
                        xht[:, kd, :],
                        start=(kd == 0),
                        stop=(kd == KD - 1),
                    )
                ps[key] = pt

            def stage1(c, j, ps):
                ef = wk.tile([128, TC], bf16, tag="ef", name=f"ef{c}_{j}")
                nc.scalar.activation(ef[:], ps["f"][:], Act.Exp, bias=b_sb["f"][:, j:j + 1], scale=-1.0)
                ei = wk.tile([128, TC], bf16, tag="ei", name=f"ei{c}_{j}")
                nc.scalar.activation(ei[:], ps["i"][:], Act.Exp, bias=b_sb["i"][:, j:j + 1], scale=-1.0)
                s2 = wk.tile([128, TC], bf16, tag="s2", name=f"s2{c}_{j}")
                nc.gpsimd.tensor_add(s2[:], ef[:], ei[:])
                pending.append((c, j, ps, ef, ei, s2))

            def stage2():
                c, j, ps, ef, ei, s2 = pending.pop(0)
                ln2 = wk.tile([128, TC], f32, tag="ln2", name=f"ln{c}_{j}")
                nc.scalar.activation(ln2[:], s2[:], Act.Ln, bias=two_sb[:, 0:1], scale=1.0)
                rt = wk.tile([128, TC], f32, tag="rt", name=f"rt{c}_{j}")
                nc.scalar.activation(rt[:], ln2[:], Act.Exp, bias=0.0, scale=-1.0)
                pending2.append((c, j, ps, ef, ei, rt))

            def stage3():
                c, j, ps, ef, ei, rt = pending2.pop(0)
                at = wk.tile([128, TC], f32, tag="a", name=f"at{c}_{j}")
                nc.vector.scalar_tensor_tensor(at[:], ei[:], 1.0, rt[:], A.add, A.mult)
                ut = wk.tile([128, TC], f32, tag="u", name=f"ut{c}_{j}")
                nc.vector.scalar_tensor_tensor(ut[:], ef[:], 1.0, rt[:], A.add, A.mult)
                bt = wk.tile([128, TC], f32, tag="b", name=f"bt{c}_{j}")
                nc.vector.scalar_tensor_tensor(bt[:], ps["h"][:], b_sb["h"][:, j:j + 1], ut[:], A.add, A.mult)
                hh = hpool.tile([128, TC], f32, tag=f"h{j}", name=f"hh{c}_{j}")
                init = h0_sb[:, j:j + 1] if c == 0 else prev_h[j][:, TC - 1:TC]
                nc.vector.tensor_tensor_scan(hh[:], at[:], bt[:], init, op0=A.mult, op1=A.add)
                prev_h[j] = hh
                nc.sync.dma_start(hT[j * 128:(j + 1) * 128, c * TC:(c + 1) * TC], hh[:])

            for c in range(NT):
                if c == 0:
                    xht = xht0
                else:
                    xht = xpool.tile([128, KD, TC], bf16, tag="xh", name=f"xh_{c}")
                    nc.sync.dma_start(xht[:], xh_d[c])

                ps_by_j = [dict() for _ in range(MH)]
                if c == 0:
                    # f-projections for all j first: they only need wf + x,
                    # streaming while wi/wh weight DMAs are still in flight.
                    for j in range(MH):
                        emit_proj(c, j, ps_by_j[j], "f", xht)
                for j in range(MH):
                    ps = ps_by_j[j]
                    if c != 0:
                        emit_proj(c, j, ps, "f", xht)
                    emit_proj(c, j, ps, "i", xht)
                    emit_proj(c, j, ps, "h", xht)
                    stage1(c, j, ps)
                    # Steady state: stage2/3 lag 1-2 groups so no in-order
                    # engine queue waits mid-chain. Last two groups: drain
                    # eagerly so the tail is one group's latency, not three.
                    lag = 1 if (c == NT - 1 and j >= MH - 2) else 2
                    while len(pending) > lag - 1:
                        stage2()
                    while len(pending2) > lag - 1:
                        stage3()
            while pending:
                stage2()
            while pending2:
                stage3()

    # Keep every ACT func in the one shared table (Exp/Ln/Identity/Copy all
    # live in "natural_log_exp_and_others"); empty the other tables so the
    # first-match table-load pass emits a single load instead of thrashing.
    import concourse.bacc as bacc_mod

    orig_tables = bacc_mod.get_activation_tables

    def _single_table(arch):
        tabs = orig_tables(arch)
        keep = "natural_log_exp_and_others"
        return {k: (v if k == keep else set()) for k, v in tabs.items()}

    bacc_mod.get_activation_tables = _single_table
    try:
        nc.compile()
    finally:
        bacc_mod.get_activation_tables = orig_tables
    return nc


def _get_nc():
    if "nc" not in _state:
        _state["nc"] = _build()
    return _state["nc"]


def _prep_inputs(x, h0, f_w, f_b, i_w, i_b, h_w, h_b):
    BF = ml_dtypes.bfloat16
    x = np.asarray(x, dtype=np.float32)
    h0 = np.asarray(h0, dtype=np.float32)
    xT = x.transpose(0, 2, 1)                                # [B, D, T]
    # xh: [B, NT, 128, KD, TC] with k = kd*128 + kp
    xh = np.ascontiguousarray(
        xT.reshape(B, KD, 128, NT, TC).transpose(0, 3, 2, 1, 4)
    ).astype(BF)
    shared = {}
    for p, w in (("f", f_w), ("i", i_w), ("h", h_w)):
        wT = np.asarray(w, dtype=np.float32).T               # [D, H]
        shared[f"w{p}"] = np.ascontiguousarray(wT.reshape(KD, 128, H)).astype(BF)
    for p, bias, sgn in (("f", f_b, -1.0), ("i", i_b, -1.0), ("h", h_b, 1.0)):
        # f/i biases negated: kernel computes Exp(-pre + bias_ap)
        bias = sgn * np.asarray(bias, dtype=np.float32)
        shared[f"b{p}"] = np.ascontiguousarray(bias.reshape(MH, 128).T)  # [128, MH]
    in_maps = []
    for b in range(B):
        m = dict(shared)
        m["xh"] = xh[b]
        m["h0c"] = np.ascontiguousarray(h0[b, 0].reshape(MH, 128).T)
        in_maps.append(m)
    return in_maps


def kernel(x, h0, f_w, f_b, i_w, i_b, h_w, h_b, _trace=False):
    from concourse.bass_utils import run_bass_kernel_spmd

    nc = _get_nc()
    in_maps = _prep_inputs(x, h0, f_w, f_b, i_w, i_b, h_w, h_b)
    res = run_bass_kernel_spmd(nc, in_maps, core_ids=list(range(B)), trace=_trace)
    out = np.empty((B, T, H), dtype=np.float32)
    for b in range(B):
        out[b] = res.results[b]["hT"].astype(np.float32).T
    if _trace:
        _state["last_results"] = res
    return out

